# revision 1
# baseline (speedup 1.0000x reference)
"""GNN message-passing kernel for trn2: preprocessing + bass/tile builder."""
import numpy as np
import ml_dtypes
import concourse.bass as bass
import concourse.tile as tile
from concourse import bacc, mybir
from concourse.bass_utils import run_bass_kernel_spmd

F32 = mybir.dt.float32
BF16 = mybir.dt.bfloat16
I16 = mybir.dt.int16
P = 128


def preprocess(x, edge_index, batch, NC=8, QUAD=32768, table_np=ml_dtypes.bfloat16):
    """Host-side graph preprocessing. Returns (struct, per_core_common, meta)."""
    x = np.asarray(x, np.float32)
    ei = np.asarray(edge_index, np.int64)
    b = np.asarray(batch, np.int64)
    N = x.shape[0]
    G = int(b.max()) + 1
    assert G % NC == 0, (G, NC)
    GPC = G // NC
    counts = np.bincount(b, minlength=G)
    assert counts.min() > 0
    WPG = int(np.ceil(counts.max() / P))  # windows per graph
    NPG = WPG * P
    WIN = GPC * WPG                      # windows per core
    NPC = WIN * P                        # padded nodes per core
    NTOT = NC * NPC
    NQ = int(np.ceil(NTOT / QUAD))

    # node permutation: graph g -> core g//GPC, slot (g%GPC)*NPG + j
    cum = np.concatenate([[0], np.cumsum(counts)])
    base_new = (np.arange(G) // GPC) * NPC + (np.arange(G) % GPC) * NPG
    perm = base_new[b] + (np.arange(N) - cum[b])     # orig id -> new id

    xt = np.zeros((NTOT, x.shape[1]), table_np)
    xt[perm] = x.astype(table_np)

    src = perm[ei[0]]
    dst = perm[ei[1]]
    deg = np.bincount(dst, minlength=NTOT)
    recip_full = (1.0 / np.maximum(deg, 1)).astype(np.float32)
    mask_full = (deg > 0).astype(np.float32)

    core = dst // NPC
    w = (dst % NPC) // P
    dl = (dst % P).astype(np.int64)
    q = src // QUAD
    key = ((core * WIN + w) * NQ + q).astype(np.int64)
    order = np.argsort(key, kind="stable")
    s_src = src[order]
    s_dl = dl[order]
    s_key = key[order]
    L = np.bincount(s_key, minlength=NC * WIN * NQ).reshape(NC, WIN, NQ)
    S = np.ceil(L / P).astype(np.int64).max(axis=0)  # [WIN, NQ] subtiles
    S_tot = int(S.sum())
    S_w = S.sum(axis=1)  # [WIN]

    # group windows for batched gathers; subtile order: (group, q, w in group)
    GRP = 4
    NG = int(np.ceil(WIN / GRP))
    sub0 = np.zeros((WIN, NQ), np.int64)
    run = 0
    for g in range(NG):
        ws = range(g * GRP, min((g + 1) * GRP, WIN))
        for qi in range(NQ):
            for wi in ws:
                sub0[wi, qi] = run
                run += S[wi, qi]
    assert run == S_tot

    # per-core edge slot arrays
    idx_flat = np.zeros((NC, S_tot * P), np.int64)       # pad -> index 0
    dl_flat = np.full((NC, S_tot * P), 255, np.int64)    # pad -> dead dst
    grp_start = np.concatenate([[0], np.cumsum(L.reshape(-1))])
    for c in range(NC):
        for wi in range(WIN):
            for qi in range(NQ):
                g = (c * WIN + wi) * NQ + qi
                n = L[c, wi, qi]
                if n == 0:
                    continue
                a = grp_start[g]
                base = sub0[wi, qi] * P
                idx_flat[c, base:base + n] = s_src[a:a + n] - qi * QUAD
                dl_flat[c, base:base + n] = s_dl[a:a + n]
    assert idx_flat.max() < QUAD and idx_flat.min() >= 0

    # wrap indices into 16 partitions: flat j -> [j%16, j//16]; replicate to 128
    idx16 = np.ascontiguousarray(
        idx_flat.reshape(NC, S_tot * 8, 16).transpose(0, 2, 1)).astype(np.int16)
    idx16 = np.tile(idx16, (1, 8, 1))                    # [NC, 128, S_tot*8]
    # dst per subtile: [128, S_tot]
    dst_arr = np.ascontiguousarray(
        dl_flat.reshape(NC, S_tot, P).transpose(0, 2, 1)).astype(ml_dtypes.bfloat16)

    recip_pc = recip_full.reshape(NC, WIN, P).transpose(0, 2, 1).copy()  # [NC,128,WIN]
    mask_pc = mask_full.reshape(NC, 1, NPC).astype(ml_dtypes.bfloat16)   # [NC,1,NPC]

    struct = dict(NC=NC, G=G, GPC=GPC, WPG=WPG, WIN=WIN, NPC=NPC, NTOT=NTOT,
                  NQ=NQ, QUAD=QUAD, S=S, sub0=sub0, S_tot=S_tot, S_w=S_w,
                  GRP=GRP, NG=NG)
    percore = dict(idx16=idx16, dst=dst_arr, recip=recip_pc, mask=mask_pc, xt=xt)
    pad_frac = S_tot * P / max(len(s_src) / NC, 1) - 1
    meta = dict(pad_frac=pad_frac, WPG=WPG, S_tot=S_tot)
    return struct, percore, meta


def build_nc(st, D=128, OUT=2, table_dt=BF16, debug_taps=False):
    NC, WIN, NPC, NTOT, NQ, QUAD = st["NC"], st["WIN"], st["NPC"], st["NTOT"], st["NQ"], st["QUAD"]
    S, sub0, S_tot, GPC, WPG = st["S"], st["sub0"], st["S_tot"], st["GPC"], st["WPG"]
    DT = BF16  # compute dtype for aggregation path

    nc = bacc.Bacc("TRN2", target_bir_lowering=False, debug=False,
                   num_devices=NC, num_swdge_queues=4,
                   dynamic_dma_scratch_size=16384)
    xt = nc.dram_tensor("xt", [NTOT, D], table_dt, kind="ExternalInput")
    idx_in = nc.dram_tensor("idx16", [P, S_tot * 8], I16, kind="ExternalInput")
    dst_in = nc.dram_tensor("dstl", [P, S_tot], BF16, kind="ExternalInput")
    recip_in = nc.dram_tensor("recip", [P, WIN], F32, kind="ExternalInput")
    mask_in = nc.dram_tensor("mask", [1, NPC], BF16, kind="ExternalInput")
    w1t_in = nc.dram_tensor("w1t", [D, D], DT, kind="ExternalInput")
    w2t_in = nc.dram_tensor("w2t", [D, D], DT, kind="ExternalInput")
    b1r_in = nc.dram_tensor("b1r", [1, D], DT, kind="ExternalInput")
    b2r_in = nc.dram_tensor("b2r", [1, D], DT, kind="ExternalInput")
    wf1t_in = nc.dram_tensor("wf1t", [D, D], F32, kind="ExternalInput")
    bf1r_in = nc.dram_tensor("bf1r", [1, D], F32, kind="ExternalInput")
    wf2t_in = nc.dram_tensor("wf2t", [D, OUT], F32, kind="ExternalInput")
    bf2r_in = nc.dram_tensor("bf2r", [1, OUT], F32, kind="ExternalInput")
    iota_in = nc.dram_tensor("iota", [P, P], BF16, kind="ExternalInput")
    identb_in = nc.dram_tensor("identb", [P, P], BF16, kind="ExternalInput")
    identf_in = nc.dram_tensor("identf", [P, P], F32, kind="ExternalInput")
    onesg_in = nc.dram_tensor("onesg", [1, NC * GPC], F32, kind="ExternalInput")
    out = nc.dram_tensor("out", [NC * GPC, OUT], F32, kind="ExternalOutput")
    if debug_taps:
        dbg_h = nc.dram_tensor("dbg_h", [NPC, D], F32, kind="ExternalOutput")
        dbg_pool = nc.dram_tensor("dbg_pool", [P, NC * GPC], F32, kind="ExternalOutput")

    with tile.TileContext(nc) as tc:
        cp = tc.alloc_tile_pool(name="const", bufs=1)
        wp = tc.alloc_tile_pool(name="work", bufs=3)
        mp = tc.alloc_tile_pool(name="msgs", bufs=2)
        ohp = tc.alloc_tile_pool(name="ohp", bufs=4)
        pp_agg = tc.alloc_tile_pool(name="ps_agg", bufs=2, space="PSUM")
        pp_t = tc.alloc_tile_pool(name="ps_t", bufs=2, space="PSUM")
        pp_h = tc.alloc_tile_pool(name="ps_h", bufs=2, space="PSUM")
        pp_p = tc.alloc_tile_pool(name="ps_p", bufs=2, space="PSUM")
        dp = tc.alloc_tile_pool(name="dram", bufs=1, space="DRAM")

        h_loc = dp.tile([NPC, D], table_dt)
        h_tab = dp.tile([NTOT, D], table_dt, addr_space="Shared")
        pag_in = dp.tile([P, GPC], F32)
        pag_out = dp.tile([NC * P, GPC], F32, addr_space="Shared")

        # constants
        idx_t = cp.tile([P, S_tot * 8], I16)
        nc.sync.dma_start(idx_t[:], idx_in[:])
        dst_t = cp.tile([P, S_tot], BF16)
        nc.sync.dma_start(dst_t[:], dst_in[:])
        recip_t = cp.tile([P, WIN], F32)
        nc.sync.dma_start(recip_t[:], recip_in[:])
        mask_t = cp.tile([1, NPC], BF16)
        nc.sync.dma_start(mask_t[:], mask_in[:])
        w1t_t = cp.tile([D, D], DT)
        nc.sync.dma_start(w1t_t[:], w1t_in[:])
        w2t_t = cp.tile([D, D], DT)
        nc.sync.dma_start(w2t_t[:], w2t_in[:])
        b1r_t = cp.tile([1, D], DT)
        nc.sync.dma_start(b1r_t[:], b1r_in[:])
        b2r_t = cp.tile([1, D], DT)
        nc.sync.dma_start(b2r_t[:], b2r_in[:])
        wf1t_t = cp.tile([D, D], F32)
        nc.sync.dma_start(wf1t_t[:], wf1t_in[:])
        bf1r_t = cp.tile([1, D], F32)
        nc.sync.dma_start(bf1r_t[:], bf1r_in[:])
        wf2t_t = cp.tile([D, OUT], F32)
        nc.sync.dma_start(wf2t_t[:], wf2t_in[:])
        bf2r_t = cp.tile([1, OUT], F32)
        nc.sync.dma_start(bf2r_t[:], bf2r_in[:])
        iota_t = cp.tile([P, P], BF16)
        nc.sync.dma_start(iota_t[:], iota_in[:])
        identb_t = cp.tile([P, P], BF16)
        nc.sync.dma_start(identb_t[:], identb_in[:])
        identf_t = cp.tile([P, P], F32)
        nc.sync.dma_start(identf_t[:], identf_in[:])
        onesg_t = cp.tile([1, NC * GPC], F32)
        nc.sync.dma_start(onesg_t[:], onesg_in[:])
        zero_t = cp.tile([P, D], table_dt)
        nc.vector.memset(zero_t[:], 0)
        poolT = cp.tile([P, GPC], F32)
        nc.vector.memset(poolT[:], 0)

        qrows = [min(QUAD, NTOT - qi * QUAD) for qi in range(NQ)]

        for layer in range(2):
            table = xt if layer == 0 else h_tab
            wt = w1t_t if layer == 0 else w2t_t
            br = b1r_t if layer == 0 else b2r_t
            GRP, NG = st["GRP"], st["NG"]
            S_grp = np.zeros((NG, NQ), np.int64)
            for g in range(NG):
                for qi in range(NQ):
                    S_grp[g, qi] = S[g * GRP:(g + 1) * GRP, qi].sum()
            gbase = np.zeros(NG, np.int64)
            for g in range(1, NG):
                gbase[g] = gbase[g - 1] + S_grp[g - 1].sum()
            Sg_max = int(S_grp.sum(axis=1).max())
            for g in range(NG):
                Sg = int(S_grp[g].sum())
                ws = list(range(g * GRP, min((g + 1) * GRP, WIN)))
                if Sg > 0:
                    msgs = mp.tile([P, Sg * D], table_dt, tag="msgs",
                                   padded_shape=[P, Sg_max * D])
                    m3 = msgs[:].rearrange("p (s d) -> p s d", d=D)
                    MAXSUB = 8  # <=1024 idx per call (16KB swdge scratch)
                    off = 0
                    for qi in range(NQ):
                        Sq = int(S_grp[g, qi])
                        done = 0
                        while done < Sq:
                            n = min(MAXSUB, Sq - done)
                            c0 = (int(gbase[g]) + off) * 8
                            nc.gpsimd.dma_gather(
                                out_ap=m3[:, off:off + n, :],
                                in_ap=table[qi * QUAD: qi * QUAD + qrows[qi], :],
                                idxs_ap=idx_t[:, c0: c0 + n * 8],
                                num_idxs=n * P,
                                num_idxs_reg=n * P,
                                elem_size=D,
                                queue_num=qi % 4,
                            )
                            off += n
                            done += n
                for w in ws:
                    Sw = int(st["S_w"][w])
                    if Sw == 0:
                        if layer == 0:
                            nc.sync.dma_start(h_loc[w * P:(w + 1) * P, :], zero_t[:])
                        continue
                    agg_p = pp_agg.tile([P, D], F32, tag="agg")
                    si = 0
                    for qi in range(NQ):
                        for s in range(int(S[w, qi])):
                            gs = int(sub0[w, qi]) + s
                            ms = gs - int(gbase[g])
                            oh = ohp.tile([P, P], BF16, tag="oh")
                            nc.vector.tensor_tensor(
                                out=oh[:],
                                in0=dst_t[:, gs:gs + 1].to_broadcast([P, P]),
                                in1=iota_t[:],
                                op=mybir.AluOpType.is_equal,
                            )
                            nc.tensor.matmul(agg_p[:], lhsT=oh[:], rhs=m3[:, ms, :],
                                             start=(si == 0), stop=(si == Sw - 1))
                            si += 1
                    agg_s = wp.tile([P, D], BF16, tag="aggs")
                    nc.vector.tensor_scalar(out=agg_s[:], in0=agg_p[:],
                                            scalar1=recip_t[:, w:w + 1], scalar2=None,
                                            op0=mybir.AluOpType.mult)
                    aggT_p = pp_t.tile([P, D], BF16, tag="aggT")
                    nc.tensor.transpose(aggT_p[:], agg_s[:], identb_t[:])
                    aggT_s = wp.tile([P, D], BF16, tag="aggTs")
                    nc.scalar.activation(aggT_s[:], aggT_p[:],
                                         mybir.ActivationFunctionType.Copy)
                    h_p = pp_h.tile([P, D], F32, tag="h")
                    nc.tensor.matmul(h_p[:], lhsT=aggT_s[:], rhs=wt[:], start=True, stop=False)
                    nc.tensor.matmul(h_p[:], lhsT=mask_t[:1, w * P:(w + 1) * P], rhs=br[:],
                                     start=False, stop=True)
                    h_s = wp.tile([P, D], table_dt, tag="hs")
                    nc.scalar.activation(h_s[:], h_p[:], mybir.ActivationFunctionType.Relu)
                    if layer == 0:
                        nc.sync.dma_start(h_loc[w * P:(w + 1) * P, :], h_s[:])
                    else:
                        lg = w // WPG
                        hT_p = pp_p.tile([P, P], BF16, tag="hT")
                        nc.tensor.transpose(hT_p[:], h_s[:], identb_t[:])
                        wmax = wp.tile([P, 1], F32, tag="wmax")
                        nc.vector.reduce_max(wmax[:], hT_p[:], axis=mybir.AxisListType.X)
                        nc.vector.tensor_tensor(out=poolT[:, lg:lg + 1], in0=wmax[:],
                                                in1=poolT[:, lg:lg + 1],
                                                op=mybir.AluOpType.max)
            if layer == 0:
                nc.gpsimd.collective_compute(
                    "AllGather", mybir.AluOpType.bypass,
                    replica_groups=[list(range(NC))],
                    ins=[h_loc[:]], outs=[h_tab[:]],
                )
        if debug_taps:
            hb = wp.tile([P, D], F32, tag="hdbg")
            for w in range(WIN):
                nc.gpsimd.dma_start(hb[:], h_loc[w * P:(w + 1) * P, :])
                nc.sync.dma_start(dbg_h[w * P:(w + 1) * P, :], hb[:])

        # ---- head ----
        NGr = NC * GPC
        nc.sync.dma_start(pag_in[:], poolT[:])
        nc.gpsimd.collective_compute(
            "AllGather", mybir.AluOpType.bypass,
            replica_groups=[list(range(NC))],
            ins=[pag_in[:]], outs=[pag_out[:]],
        )
        pall = cp.tile([P, NGr], F32)
        pr = pag_out[:].rearrange("(c p) g -> p c g", c=NC)
        for c in range(NC):
            nc.sync.dma_start(pall[:, c * GPC:(c + 1) * GPC], pr[:, c, :])
        if debug_taps:
            nc.sync.dma_start(dbg_pool[:], pall[:])
        z_p = pp_agg.tile([P, NGr], F32, tag="agg")
        nc.tensor.matmul(z_p[:], lhsT=wf1t_t[:], rhs=pall[:], start=True, stop=False)
        nc.tensor.matmul(z_p[:], lhsT=bf1r_t[:1, :], rhs=onesg_t[:1, :], start=False, stop=True)
        zr = wp.tile([P, NGr], F32, tag="zr")
        nc.scalar.activation(zr[:], z_p[:], mybir.ActivationFunctionType.Relu)
        z2_p = pp_h.tile([OUT, NGr], F32, tag="h")
        nc.tensor.matmul(z2_p[:], lhsT=wf2t_t[:], rhs=zr[:], start=True, stop=False)
        nc.tensor.matmul(z2_p[:], lhsT=bf2r_t[:1, :], rhs=onesg_t[:1, :], start=False, stop=True)
        z2 = wp.tile([OUT, NGr], F32, tag="z2")
        nc.vector.tensor_copy(z2[:], z2_p[:])
        zt_p = pp_t.tile([NGr, OUT], F32, tag="aggT")
        nc.tensor.transpose(zt_p[:], z2[:], identf_t[:OUT, :OUT])
        zt = wp.tile([NGr, OUT], F32, tag="zt")
        nc.vector.tensor_copy(zt[:], zt_p[:])
        mx = wp.tile([NGr, 1], F32, tag="mx")
        nc.vector.reduce_max(mx[:], zt[:], axis=mybir.AxisListType.X)
        zs = wp.tile([NGr, OUT], F32, tag="zs")
        nc.vector.tensor_scalar(out=zs[:], in0=zt[:], scalar1=mx[:], scalar2=None,
                                op0=mybir.AluOpType.subtract)
        ex = wp.tile([NGr, OUT], F32, tag="ex")
        nc.scalar.activation(ex[:], zs[:], mybir.ActivationFunctionType.Exp)
        sm = wp.tile([NGr, 1], F32, tag="sm")
        nc.vector.reduce_sum(sm[:], ex[:], axis=mybir.AxisListType.X)
        lg_ = wp.tile([NGr, 1], F32, tag="lg")
        nc.scalar.activation(lg_[:], sm[:], mybir.ActivationFunctionType.Ln)
        logz = wp.tile([NGr, 1], F32, tag="logz")
        nc.vector.tensor_tensor(out=logz[:], in0=mx[:], in1=lg_[:],
                                op=mybir.AluOpType.add)
        res = wp.tile([NGr, OUT], F32, tag="res")
        nc.vector.tensor_scalar(out=res[:], in0=zt[:], scalar1=logz[:], scalar2=None,
                                op0=mybir.AluOpType.subtract)
        nc.sync.dma_start(out[:], res[:])

        for p_ in (dp, pp_p, pp_h, pp_t, pp_agg, ohp, mp, wp, cp):
            p_.release()
    nc.compile()
    return nc


def make_inputs(st, percore, W1, b1, W2, b2, Wf1, bf1, Wf2, bf2, table_np=np.float32):
    NC, GPC = st["NC"], st["GPC"]
    bf = ml_dtypes.bfloat16
    iota = np.broadcast_to(np.arange(P, dtype=np.float32), (P, P)).astype(bf)
    ident = np.eye(P, dtype=np.float32)
    common = dict(
        xt=percore["xt"],
        w1t=np.ascontiguousarray(np.asarray(W1, np.float32).T).astype(bf),
        w2t=np.ascontiguousarray(np.asarray(W2, np.float32).T).astype(bf),
        b1r=np.asarray(b1, np.float32)[None, :].astype(bf),
        b2r=np.asarray(b2, np.float32)[None, :].astype(bf),
        wf1t=np.ascontiguousarray(np.asarray(Wf1, np.float32).T),
        bf1r=np.asarray(bf1, np.float32)[None, :],
        wf2t=np.ascontiguousarray(np.asarray(Wf2, np.float32).T),
        bf2r=np.asarray(bf2, np.float32)[None, :],
        iota=np.ascontiguousarray(iota),
        identb=ident.astype(bf),
        identf=ident,
        onesg=np.ones((1, NC * GPC), np.float32),
    )
    in_maps = []
    for c in range(NC):
        m = dict(common)
        m["idx16"] = np.ascontiguousarray(percore["idx16"][c])
        m["dstl"] = np.ascontiguousarray(percore["dst"][c])
        m["recip"] = np.ascontiguousarray(percore["recip"][c])
        m["mask"] = np.ascontiguousarray(percore["mask"][c])
        in_maps.append(m)
    return in_maps


_CACHE = {}


def kernel(**inputs):
    """Full-input GNN kernel: shards across 8 NeuronCores internally."""
    import os
    x = np.asarray(inputs["x"], np.float32)
    ei = np.asarray(inputs["edge_index"])
    batch = np.asarray(inputs["batch"])
    st, percore, _meta = preprocess(x, ei, batch)
    key = (st["WIN"], st["NPC"], st["S_tot"], st["NQ"])
    if key not in _CACHE:
        _CACHE[key] = build_nc(st)
    nc = _CACHE[key]
    in_maps = make_inputs(st, percore,
                          inputs["W1"], inputs["b1"], inputs["W2"], inputs["b2"],
                          inputs["Wf1"], inputs["bf1"], inputs["Wf2"], inputs["bf2"])
    trace = os.environ.get("GNN_TRACE", "0") == "1"
    res = run_bass_kernel_spmd(nc, in_maps, core_ids=list(range(st["NC"])), trace=trace)
    global LAST_EXEC_NS, LAST_TRACE
    LAST_EXEC_NS = res.exec_time_ns
    LAST_TRACE = res.instructions_and_trace[1] if res.instructions_and_trace else None
    return np.asarray(res.results[0]["out"], np.float32)


LAST_EXEC_NS = None
LAST_TRACE = None



# revision 2
# speedup vs baseline: 1.1770x; 1.1770x over previous
"""GNN message-passing kernel for trn2: deep-buffered edge gathers + quad-major one-hot scatter."""
import os as _os
import numpy as np
import ml_dtypes
NOCOMPUTE = _os.environ.get("GNN_NOCOMPUTE", "0") == "1"
import concourse.bass as bass
import concourse.tile as tile
from concourse import bacc, mybir
from concourse.bass_utils import run_bass_kernel_spmd

F32 = mybir.dt.float32
BF16 = mybir.dt.bfloat16
I16 = mybir.dt.int16
P = 128
NC = 8
MAXSUB = 8          # subtiles per gather call (1024 idx)
NCHUNK = 4          # allgather chunks == quads


def preprocess(x, edge_index, batch):
    """Host-side graph preprocessing.

    Node layout is chunk-major: global id = chunk*QUAD + core*CPN + local,
    so each allgather chunk is contiguous and one chunk == one src quad.
    """
    x = np.asarray(x, np.float32)
    ei = np.asarray(edge_index, np.int64)
    b = np.asarray(batch, np.int64)
    N = x.shape[0]
    G = int(b.max()) + 1
    GPC = G // NC
    counts = np.bincount(b, minlength=G)
    WPG = int(np.ceil(counts.max() / P))
    NPG = WPG * P
    WIN = GPC * WPG                 # windows per core
    NPC = WIN * P                   # padded nodes per core
    NTOT = NC * NPC
    assert WIN % NCHUNK == 0, WIN
    WCH = WIN // NCHUNK             # windows per chunk
    CPN = WCH * P                   # nodes per chunk per core
    QUAD = NC * CPN                 # nodes per global chunk (= src quad)
    assert QUAD <= 32768, QUAD
    NQ = NCHUNK

    # local node id within core: graph-major with per-graph padding
    cum = np.concatenate([[0], np.cumsum(counts)])
    core_of_g = np.arange(G) // GPC
    base_loc = (np.arange(G) % GPC) * NPG
    loc = base_loc[b] + (np.arange(N) - cum[b])      # local id in [0, NPC)
    core = core_of_g[b]
    # global id: chunk-major
    ch = loc // CPN
    glob = ch * QUAD + core * CPN + (loc % CPN)

    xt = np.zeros((NTOT, x.shape[1]), ml_dtypes.bfloat16)
    xt[glob] = x.astype(ml_dtypes.bfloat16)

    src = glob[ei[0]]
    dst = glob[ei[1]]
    # destination core / local / window / lane
    dch = dst // QUAD
    drem = dst % QUAD
    dcore = drem // CPN
    dloc = dch * CPN + (drem % CPN)                  # local node id on dcore
    w = dloc // P
    dl = dloc % P
    q = src // QUAD

    deg = np.zeros(NC * NPC, np.int64)
    np.add.at(deg, dcore * NPC + dloc, 1)
    deg = deg.reshape(NC, NPC)
    recip_full = (1.0 / np.maximum(deg, 1)).astype(np.float32)
    mask_full = (deg > 0).astype(ml_dtypes.bfloat16)

    # bucket edges by (core, q, w); subtile counts maxed over cores (SPMD)
    L = np.zeros((NC, NQ, WIN), np.int64)
    np.add.at(L, (dcore, q, w), 1)
    S = np.ceil(L / P).astype(np.int64).max(axis=0)  # [NQ, WIN]
    assert S.max() <= 8, S.max()
    assert S.min() >= 1, S.min()  # every (quad, window) has edges
    S_tot = int(S.sum())
    sub0 = np.zeros((NQ, WIN), np.int64)             # first subtile of (q, w)
    run = 0
    for qi in range(NQ):
        for wi in range(WIN):
            sub0[qi, wi] = run
            run += S[qi, wi]
    assert run == S_tot

    order = np.argsort(dcore * (NQ * WIN) + q * WIN + w, kind="stable")
    s_src = src[order]
    s_dl = dl[order]
    s_q = q[order]
    s_core = dcore[order]
    bstart = np.concatenate([[0], np.cumsum(L.transpose(0, 1, 2).reshape(-1))])

    idx_flat = np.zeros((NC, S_tot * P), np.int64)
    dl_flat = np.full((NC, S_tot * P), 255, np.int64)
    for c in range(NC):
        for qi in range(NQ):
            for wi in range(WIN):
                gidx = (c * NQ + qi) * WIN + wi
                n = L[c, qi, wi]
                if n == 0:
                    continue
                a = bstart[gidx]
                base = sub0[qi, wi] * P
                idx_flat[c, base:base + n] = s_src[a:a + n] - qi * QUAD
                dl_flat[c, base:base + n] = s_dl[a:a + n]
    assert idx_flat.max() < QUAD and idx_flat.min() >= 0

    idx16 = np.ascontiguousarray(
        idx_flat.reshape(NC, S_tot * 8, 16).transpose(0, 2, 1)).astype(np.int16)
    idx16 = np.tile(idx16, (1, 8, 1))                     # [NC, 128, S_tot*8]
    dst_arr = np.ascontiguousarray(
        dl_flat.reshape(NC, S_tot, P).transpose(0, 2, 1)).astype(ml_dtypes.bfloat16)

    recip_rep = np.broadcast_to(recip_full.astype(ml_dtypes.bfloat16)[:, None, :],
                                (NC, P, NPC)).copy()               # [NC,128,NPC]
    mask_pc = mask_full.reshape(NC, 1, NPC)

    st = dict(NC=NC, G=G, GPC=GPC, WPG=WPG, WIN=WIN, WCH=WCH, CPN=CPN,
              NPC=NPC, NTOT=NTOT, NQ=NQ, QUAD=QUAD, S=S, sub0=sub0,
              S_tot=S_tot)
    percore = dict(idx16=idx16, dst=dst_arr, recip=recip_rep, mask=mask_pc, xt=xt)
    return st, percore


def build_nc(st, D=128, OUT=2):
    WIN, NPC, NTOT, NQ, QUAD = st["WIN"], st["NPC"], st["NTOT"], st["NQ"], st["QUAD"]
    S, sub0, S_tot, GPC, WPG = st["S"], st["sub0"], st["S_tot"], st["GPC"], st["WPG"]
    WCH, CPN = st["WCH"], st["CPN"]

    nc = bacc.Bacc("TRN2", target_bir_lowering=False, debug=False,
                   num_devices=NC, num_swdge_queues=4,
                   dynamic_dma_scratch_size=65536)
    xt = nc.dram_tensor("xt", [NTOT, D], BF16, kind="ExternalInput")
    idx_in = nc.dram_tensor("idx16", [P, S_tot * 8], I16, kind="ExternalInput")
    dst_in = nc.dram_tensor("dstl", [P, S_tot], BF16, kind="ExternalInput")
    recip_in = nc.dram_tensor("recip", [P, NPC], BF16, kind="ExternalInput")
    mask_in = nc.dram_tensor("mask", [1, NPC], BF16, kind="ExternalInput")
    w1t_in = nc.dram_tensor("w1t", [D, D], BF16, kind="ExternalInput")
    w2t_in = nc.dram_tensor("w2t", [D, D], BF16, kind="ExternalInput")
    b1r_in = nc.dram_tensor("b1r", [1, D], BF16, kind="ExternalInput")
    b2r_in = nc.dram_tensor("b2r", [1, D], BF16, kind="ExternalInput")
    wf1t_in = nc.dram_tensor("wf1t", [D, D], F32, kind="ExternalInput")
    bf1r_in = nc.dram_tensor("bf1r", [1, D], F32, kind="ExternalInput")
    wf2t_in = nc.dram_tensor("wf2t", [D, OUT], F32, kind="ExternalInput")
    bf2r_in = nc.dram_tensor("bf2r", [1, OUT], F32, kind="ExternalInput")
    iota_in = nc.dram_tensor("iota", [P, P], BF16, kind="ExternalInput")
    identf_in = nc.dram_tensor("identf", [P, P], F32, kind="ExternalInput")
    onesg_in = nc.dram_tensor("onesg", [1, NC * GPC], F32, kind="ExternalInput")
    out = nc.dram_tensor("out", [NC * GPC, OUT], F32, kind="ExternalOutput")

    with tile.TileContext(nc) as tc:
        cp = tc.alloc_tile_pool(name="const", bufs=1)
        mp = tc.alloc_tile_pool(name="msgs", bufs=10)
        ohp = tc.alloc_tile_pool(name="ohp", bufs=4)
        wp = tc.alloc_tile_pool(name="work", bufs=3)
        pp_agg = tc.alloc_tile_pool(name="ps_agg", bufs=4, space="PSUM")
        pp_h = tc.alloc_tile_pool(name="ps_h", bufs=2, space="PSUM")
        dp = tc.alloc_tile_pool(name="dram", bufs=1, space="DRAM")

        h_loc = dp.tile([NPC, D], BF16)
        h_tabs = [dp.tile([QUAD, D], BF16, addr_space="Shared", name=f"htab{c}")
                  for c in range(NQ)]
        pag_in = dp.tile([P, GPC], F32)
        pag_out = dp.tile([NC * P, GPC], F32, addr_space="Shared")

        idx_t = cp.tile([P, S_tot * 8], I16)
        nc.sync.dma_start(idx_t[:], idx_in[:])
        dst_t = cp.tile([P, S_tot], BF16)
        nc.sync.dma_start(dst_t[:], dst_in[:])
        recip_t = cp.tile([P, NPC], BF16)
        nc.sync.dma_start(recip_t[:], recip_in[:])
        mask_t = cp.tile([1, NPC], BF16)
        nc.sync.dma_start(mask_t[:], mask_in[:])
        w1t_t = cp.tile([D, D], BF16)
        nc.sync.dma_start(w1t_t[:], w1t_in[:])
        w2t_t = cp.tile([D, D], BF16)
        nc.sync.dma_start(w2t_t[:], w2t_in[:])
        b1r_t = cp.tile([1, D], BF16)
        nc.sync.dma_start(b1r_t[:], b1r_in[:])
        b2r_t = cp.tile([1, D], BF16)
        nc.sync.dma_start(b2r_t[:], b2r_in[:])
        wf1t_t = cp.tile([D, D], F32)
        nc.sync.dma_start(wf1t_t[:], wf1t_in[:])
        bf1r_t = cp.tile([1, D], F32)
        nc.sync.dma_start(bf1r_t[:], bf1r_in[:])
        wf2t_t = cp.tile([D, OUT], F32)
        nc.sync.dma_start(wf2t_t[:], wf2t_in[:])
        bf2r_t = cp.tile([1, OUT], F32)
        nc.sync.dma_start(bf2r_t[:], bf2r_in[:])
        iota_t = cp.tile([P, P], BF16)
        nc.sync.dma_start(iota_t[:], iota_in[:])
        identf_t = cp.tile([P, P], F32)
        nc.sync.dma_start(identf_t[:], identf_in[:])
        onesg_t = cp.tile([1, NC * GPC], F32)
        nc.sync.dma_start(onesg_t[:], onesg_in[:])
        poolT = cp.tile([P, GPC], F32)
        nc.vector.memset(poolT[:], 0)
        # SBUF f32 window accumulators, transposed layout [feat, 104*128 dst]
        aggsb = cp.tile([P, WIN * P], BF16)

        # per-quad call schedule: subtiles sub0[q,0] .. sub0[q,0]+Sq
        qstart = [int(sub0[qi, 0]) for qi in range(NQ)]
        qcount = [int(S[qi].sum()) for qi in range(NQ)]

        call_no = [0]

        def epilogue(w, layer):
            """Normalize window w, apply weights; L1 -> h rows; L2 -> pool."""
            a = aggsb[:, w * P:(w + 1) * P]
            an = wp.tile([P, P], BF16, tag="an")
            nc.vector.tensor_tensor(
                out=an[:], in0=a,
                in1=recip_t[:, w * P:(w + 1) * P],
                op=mybir.AluOpType.mult)
            h_p = pp_h.tile([P, D], F32, tag="h")
            if layer == 0:
                nc.tensor.matmul(h_p[:], lhsT=an[:], rhs=w1t_t[:], start=True, stop=False)
                nc.tensor.matmul(h_p[:], lhsT=mask_t[:1, w * P:(w + 1) * P],
                                 rhs=b1r_t[:1, :], start=False, stop=True)
                h_s = wp.tile([P, D], BF16, tag="hs")
                nc.scalar.activation(h_s[:], h_p[:], mybir.ActivationFunctionType.Relu)
                nc.sync.dma_start(h_loc[w * P:(w + 1) * P, :], h_s[:])
            else:
                nc.tensor.matmul(h_p[:], lhsT=w2t_t[:], rhs=an[:], start=True, stop=False)
                nc.tensor.matmul(h_p[:], lhsT=b2r_t[:1, :],
                                 rhs=mask_t[:1, w * P:(w + 1) * P], start=False, stop=True)
                h_s = wp.tile([P, D], BF16, tag="hs")
                nc.scalar.activation(h_s[:], h_p[:], mybir.ActivationFunctionType.Relu)
                lg = w // WPG
                wmax = wp.tile([P, 1], F32, tag="wmax")
                nc.vector.reduce_max(wmax[:], h_s[:], axis=mybir.AxisListType.X)
                nc.vector.tensor_tensor(out=poolT[:, lg:lg + 1], in0=wmax[:],
                                        in1=poolT[:, lg:lg + 1],
                                        op=mybir.AluOpType.max)

        def run_layer(table, layer):
            for qi in range(NQ):
                # subtile schedule of this quad: list of (w, s_in_window, gs)
                sched = []
                for w in range(WIN):
                    for si in range(int(S[qi, w])):
                        sched.append((w, si, int(sub0[qi, w]) + si))
                # zero/epilogue for empty windows
                for w in range(WIN):
                    if int(S[qi, w]) == 0:
                        if qi == 0:
                            nc.vector.memset(aggsb[:, w * P:(w + 1) * P], 0)
                        if qi == NQ - 1:
                            epilogue(w, layer)
                pos = 0
                oh_cur = [None, -1]  # (o3 view, window)
                agg_cur = [None]
                pend = [None]

                def finish_window(w, agg_p):
                    dstslice = aggsb[:, w * P:(w + 1) * P]
                    if qi == 0:
                        nc.vector.tensor_copy(dstslice, agg_p[:])
                    else:
                        nc.vector.tensor_tensor(out=dstslice, in0=agg_p[:],
                                                in1=dstslice,
                                                op=mybir.AluOpType.add)
                    if qi == NQ - 1:
                        epilogue(w, layer)

                while pos < len(sched):
                    n = min(MAXSUB, len(sched) - pos)
                    gs0 = sched[pos][2]
                    mg = mp.tile([P, MAXSUB * D], BF16, tag="msgs")
                    m3 = mg[:].rearrange("p (s d) -> p s d", d=D)
                    tab_ap = (table[qi * QUAD:(qi + 1) * QUAD, :]
                              if layer == 0 else h_tabs[qi][:, :])
                    nc.gpsimd.dma_gather(
                        out_ap=m3[:, :n, :],
                        in_ap=tab_ap,
                        idxs_ap=idx_t[:, gs0 * 8: (gs0 + n) * 8],
                        num_idxs=n * P,
                        num_idxs_reg=n * P,
                        elem_size=D,
                        single_packet=False,
                        queue_num=call_no[0] % 4,
                    )
                    call_no[0] += 1
                    if NOCOMPUTE:
                        pos += n
                        continue
                    for j in range(n):
                        w, si, gs = sched[pos + j]
                        Sqw = int(S[qi, w])
                        if si == 0:
                            g0 = int(sub0[qi, w])
                            oh = ohp.tile([P, Sqw * P], BF16, tag="oh",
                                          padded_shape=[P, MAXSUB * P])
                            o3 = oh[:].rearrange("p (s c) -> p s c", c=P)
                            nc.vector.tensor_tensor(
                                out=o3[:, :, :],
                                in0=dst_t[:, g0:g0 + Sqw].rearrange("p (s one) -> p s one", one=1)
                                    .to_broadcast([P, Sqw, P]),
                                in1=iota_t[:].rearrange("p (s c) -> p s c", c=P)
                                    .to_broadcast([P, Sqw, P]),
                                op=mybir.AluOpType.is_equal)
                            # flush/epilogue of the PREVIOUS window goes out
                            # after this one-hot: breaks the DVE->PE ping-pong
                            if pend[0] is not None:
                                finish_window(*pend[0])
                                pend[0] = None
                            oh_cur[0], oh_cur[1] = o3, w
                            agg_cur[0] = pp_agg.tile([P, P], F32, tag="agg", name="aggp")
                        o3w = oh_cur[0]
                        agg_p = agg_cur[0]
                        nc.tensor.matmul(agg_p[:], lhsT=m3[:, j, :], rhs=o3w[:, si, :],
                                         start=(si == 0), stop=(si == Sqw - 1))
                        if si == Sqw - 1:
                            pend[0] = (w, agg_p)
                    pos += n
                if pend[0] is not None:
                    finish_window(*pend[0])
                    pend[0] = None

        for layer in range(2):
            table = xt
            run_layer(table, layer)
            if layer == 0:
                for c in range(NQ):
                    nc.gpsimd.collective_compute(
                        "AllGather", mybir.AluOpType.bypass,
                        replica_groups=[list(range(NC))],
                        ins=[h_loc[c * CPN:(c + 1) * CPN, :]],
                        outs=[h_tabs[c][:, :]],
                    )

        # ---- head ----
        NGr = NC * GPC
        nc.sync.dma_start(pag_in[:], poolT[:])
        nc.gpsimd.collective_compute(
            "AllGather", mybir.AluOpType.bypass,
            replica_groups=[list(range(NC))],
            ins=[pag_in[:]], outs=[pag_out[:]],
        )
        pall = cp.tile([P, NGr], F32)
        pr = pag_out[:].rearrange("(c p) g -> p c g", c=NC)
        for c in range(NC):
            nc.sync.dma_start(pall[:, c * GPC:(c + 1) * GPC], pr[:, c, :])
        z_p = pp_agg.tile([P, NGr], F32, tag="agg")
        nc.tensor.matmul(z_p[:], lhsT=wf1t_t[:], rhs=pall[:], start=True, stop=False)
        nc.tensor.matmul(z_p[:], lhsT=bf1r_t[:1, :], rhs=onesg_t[:1, :], start=False, stop=True)
        zr = wp.tile([P, NGr], F32, tag="zr")
        nc.scalar.activation(zr[:], z_p[:], mybir.ActivationFunctionType.Relu)
        z2_p = pp_h.tile([OUT, NGr], F32, tag="h")
        nc.tensor.matmul(z2_p[:], lhsT=wf2t_t[:], rhs=zr[:], start=True, stop=False)
        nc.tensor.matmul(z2_p[:], lhsT=bf2r_t[:1, :], rhs=onesg_t[:1, :], start=False, stop=True)
        z2 = wp.tile([OUT, NGr], F32, tag="z2")
        nc.vector.tensor_copy(z2[:], z2_p[:])
        zt_p = pp_agg.tile([NGr, OUT], F32, tag="agg")
        nc.tensor.transpose(zt_p[:], z2[:], identf_t[:OUT, :OUT])
        zt = wp.tile([NGr, OUT], F32, tag="zt")
        nc.vector.tensor_copy(zt[:], zt_p[:])
        mx = wp.tile([NGr, 1], F32, tag="mx")
        nc.vector.reduce_max(mx[:], zt[:], axis=mybir.AxisListType.X)
        zs = wp.tile([NGr, OUT], F32, tag="zs")
        nc.vector.tensor_scalar(out=zs[:], in0=zt[:], scalar1=mx[:], scalar2=None,
                                op0=mybir.AluOpType.subtract)
        ex = wp.tile([NGr, OUT], F32, tag="ex")
        nc.scalar.activation(ex[:], zs[:], mybir.ActivationFunctionType.Exp)
        sm = wp.tile([NGr, 1], F32, tag="sm")
        nc.vector.reduce_sum(sm[:], ex[:], axis=mybir.AxisListType.X)
        lg_ = wp.tile([NGr, 1], F32, tag="lg")
        nc.scalar.activation(lg_[:], sm[:], mybir.ActivationFunctionType.Ln)
        logz = wp.tile([NGr, 1], F32, tag="logz")
        nc.vector.tensor_tensor(out=logz[:], in0=mx[:], in1=lg_[:],
                                op=mybir.AluOpType.add)
        res = wp.tile([NGr, OUT], F32, tag="res")
        nc.vector.tensor_scalar(out=res[:], in0=zt[:], scalar1=logz[:], scalar2=None,
                                op0=mybir.AluOpType.subtract)
        nc.sync.dma_start(out[:], res[:])

        for p_ in (dp, pp_h, pp_agg, wp, ohp, mp, cp):
            p_.release()
    nc.compile()
    return nc


def make_inputs(st, percore, W1, b1, W2, b2, Wf1, bf1, Wf2, bf2):
    GPC = st["GPC"]
    bf = ml_dtypes.bfloat16
    iota = np.broadcast_to(np.arange(P, dtype=np.float32), (P, P)).astype(bf)
    ident = np.eye(P, dtype=np.float32)
    common = dict(
        xt=percore["xt"],
        w1t=np.ascontiguousarray(np.asarray(W1, np.float32).T).astype(bf),
        w2t=np.ascontiguousarray(np.asarray(W2, np.float32).T).astype(bf),
        b1r=np.asarray(b1, np.float32)[None, :].astype(bf),
        b2r=np.asarray(b2, np.float32)[None, :].astype(bf),
        wf1t=np.ascontiguousarray(np.asarray(Wf1, np.float32).T),
        bf1r=np.asarray(bf1, np.float32)[None, :],
        wf2t=np.ascontiguousarray(np.asarray(Wf2, np.float32).T),
        bf2r=np.asarray(bf2, np.float32)[None, :],
        iota=np.ascontiguousarray(iota),
        identf=ident,
        onesg=np.ones((1, NC * GPC), np.float32),
    )
    in_maps = []
    for c in range(NC):
        m = dict(common)
        m["idx16"] = np.ascontiguousarray(percore["idx16"][c])
        m["dstl"] = np.ascontiguousarray(percore["dst"][c])
        m["recip"] = np.ascontiguousarray(percore["recip"][c])
        m["mask"] = np.ascontiguousarray(percore["mask"][c])
        in_maps.append(m)
    return in_maps


_CACHE = {}


def kernel(**inputs):
    """Full-input GNN kernel: shards across 8 NeuronCores internally."""
    import os
    x = np.asarray(inputs["x"], np.float32)
    ei = np.asarray(inputs["edge_index"])
    batch = np.asarray(inputs["batch"])
    st, percore = preprocess(x, ei, batch)
    key = (st["WIN"], st["NPC"], st["S_tot"], st["NQ"])
    if key not in _CACHE:
        _CACHE[key] = build_nc(st)
    nc = _CACHE[key]
    in_maps = make_inputs(st, percore,
                          inputs["W1"], inputs["b1"], inputs["W2"], inputs["b2"],
                          inputs["Wf1"], inputs["bf1"], inputs["Wf2"], inputs["bf2"])
    trace = os.environ.get("GNN_TRACE", "0") == "1"
    res = run_bass_kernel_spmd(nc, in_maps, core_ids=list(range(NC)), trace=trace)
    global LAST_EXEC_NS, LAST_TRACE
    LAST_EXEC_NS = res.exec_time_ns
    LAST_TRACE = res.instructions_and_trace[1] if res.instructions_and_trace else None
    return np.asarray(res.results[0]["out"], np.float32)


LAST_EXEC_NS = None
LAST_TRACE = None


# revision 3
# speedup vs baseline: 1.2354x; 1.0497x over previous
"""GNN message-passing kernel for trn2: deep-buffered edge gathers + quad-major one-hot scatter."""
import os as _os
import numpy as np
import ml_dtypes
NOCOMPUTE = _os.environ.get("GNN_NOCOMPUTE", "0") == "1"
import concourse.bass as bass
import concourse.tile as tile
from concourse import bacc, mybir
from concourse.bass_utils import run_bass_kernel_spmd

F32 = mybir.dt.float32
BF16 = mybir.dt.bfloat16
I16 = mybir.dt.int16
P = 128
NC = 8
MAXSUB = 8          # subtiles per gather call (1024 idx)
NCHUNK = 4          # allgather chunks == quads


def preprocess(x, edge_index, batch):
    """Host-side graph preprocessing.

    Node layout is chunk-major: global id = chunk*QUAD + core*CPN + local,
    so each allgather chunk is contiguous and one chunk == one src quad.
    """
    x = np.asarray(x, np.float32)
    ei = np.asarray(edge_index, np.int64)
    b = np.asarray(batch, np.int64)
    N = x.shape[0]
    G = int(b.max()) + 1
    GPC = G // NC
    counts = np.bincount(b, minlength=G)
    WPG = int(np.ceil(counts.max() / P))
    NPG = WPG * P
    WIN = GPC * WPG                 # windows per core
    NPC = WIN * P                   # padded nodes per core
    NTOT = NC * NPC
    assert WIN % NCHUNK == 0, WIN
    WCH = WIN // NCHUNK             # windows per chunk
    CPN = WCH * P                   # nodes per chunk per core
    QUAD = NC * CPN                 # nodes per global chunk (= src quad)
    assert QUAD <= 32768, QUAD
    NQ = NCHUNK

    # local node id within core: graph-major with per-graph padding
    cum = np.concatenate([[0], np.cumsum(counts)])
    core_of_g = np.arange(G) // GPC
    base_loc = (np.arange(G) % GPC) * NPG
    loc = base_loc[b] + (np.arange(N) - cum[b])      # local id in [0, NPC)
    core = core_of_g[b]
    # global id: chunk-major
    ch = loc // CPN
    glob = ch * QUAD + core * CPN + (loc % CPN)

    xt = np.zeros((NTOT, x.shape[1]), ml_dtypes.bfloat16)
    xt[glob] = x.astype(ml_dtypes.bfloat16)

    src = glob[ei[0]]
    dst = glob[ei[1]]
    # destination core / local / window / lane
    dch = dst // QUAD
    drem = dst % QUAD
    dcore = drem // CPN
    dloc = dch * CPN + (drem % CPN)                  # local node id on dcore
    w = dloc // P
    dl = dloc % P
    q = src // QUAD

    deg = np.zeros(NC * NPC, np.int64)
    np.add.at(deg, dcore * NPC + dloc, 1)
    deg = deg.reshape(NC, NPC)
    recip_full = (1.0 / np.maximum(deg, 1)).astype(np.float32)
    mask_full = (deg > 0).astype(ml_dtypes.bfloat16)

    # bucket edges by (core, q, w); subtile counts maxed over cores (SPMD)
    L = np.zeros((NC, NQ, WIN), np.int64)
    np.add.at(L, (dcore, q, w), 1)
    S = np.ceil(L / P).astype(np.int64).max(axis=0)  # [NQ, WIN]
    assert S.max() <= 8, S.max()
    assert S.min() >= 1, S.min()  # every (quad, window) has edges
    S_tot = int(S.sum())
    sub0 = np.zeros((NQ, WIN), np.int64)             # first subtile of (q, w)
    run = 0
    for qi in range(NQ):
        for wi in range(WIN):
            sub0[qi, wi] = run
            run += S[qi, wi]
    assert run == S_tot

    order = np.argsort(dcore * (NQ * WIN) + q * WIN + w, kind="stable")
    s_src = src[order]
    s_dl = dl[order]
    s_q = q[order]
    s_core = dcore[order]
    bstart = np.concatenate([[0], np.cumsum(L.transpose(0, 1, 2).reshape(-1))])

    idx_flat = np.zeros((NC, S_tot * P), np.int64)
    dl_flat = np.full((NC, S_tot * P), 255, np.int64)
    for c in range(NC):
        for qi in range(NQ):
            for wi in range(WIN):
                gidx = (c * NQ + qi) * WIN + wi
                n = L[c, qi, wi]
                if n == 0:
                    continue
                a = bstart[gidx]
                base = sub0[qi, wi] * P
                idx_flat[c, base:base + n] = s_src[a:a + n] - qi * QUAD
                dl_flat[c, base:base + n] = s_dl[a:a + n]
    assert idx_flat.max() < QUAD and idx_flat.min() >= 0

    idx16 = np.ascontiguousarray(
        idx_flat.reshape(NC, S_tot * 8, 16).transpose(0, 2, 1)).astype(np.int16)
    idx16 = np.tile(idx16, (1, 8, 1))                     # [NC, 128, S_tot*8]
    dst_arr = np.ascontiguousarray(
        dl_flat.reshape(NC, S_tot, P).transpose(0, 2, 1)).astype(ml_dtypes.bfloat16)

    recip_rep = np.broadcast_to(recip_full.astype(ml_dtypes.bfloat16)[:, None, :],
                                (NC, P, NPC)).copy()               # [NC,128,NPC]
    mask_pc = mask_full.reshape(NC, 1, NPC)

    st = dict(NC=NC, G=G, GPC=GPC, WPG=WPG, WIN=WIN, WCH=WCH, CPN=CPN,
              NPC=NPC, NTOT=NTOT, NQ=NQ, QUAD=QUAD, S=S, sub0=sub0,
              S_tot=S_tot)
    percore = dict(idx16=idx16, dst=dst_arr, recip=recip_rep, mask=mask_pc, xt=xt)
    return st, percore


def build_nc(st, D=128, OUT=2):
    WIN, NPC, NTOT, NQ, QUAD = st["WIN"], st["NPC"], st["NTOT"], st["NQ"], st["QUAD"]
    S, sub0, S_tot, GPC, WPG = st["S"], st["sub0"], st["S_tot"], st["GPC"], st["WPG"]
    WCH, CPN = st["WCH"], st["CPN"]

    nc = bacc.Bacc("TRN2", target_bir_lowering=False, debug=False,
                   num_devices=NC, num_swdge_queues=4,
                   dynamic_dma_scratch_size=65536)
    xt = nc.dram_tensor("xt", [NTOT, D], BF16, kind="ExternalInput")
    idx_in = nc.dram_tensor("idx16", [P, S_tot * 8], I16, kind="ExternalInput")
    dst_in = nc.dram_tensor("dstl", [P, S_tot], BF16, kind="ExternalInput")
    recip_in = nc.dram_tensor("recip", [P, NPC], BF16, kind="ExternalInput")
    mask_in = nc.dram_tensor("mask", [1, NPC], BF16, kind="ExternalInput")
    w1t_in = nc.dram_tensor("w1t", [D, D], BF16, kind="ExternalInput")
    w2t_in = nc.dram_tensor("w2t", [D, D], BF16, kind="ExternalInput")
    b1r_in = nc.dram_tensor("b1r", [1, D], BF16, kind="ExternalInput")
    b2r_in = nc.dram_tensor("b2r", [1, D], BF16, kind="ExternalInput")
    wf1t_in = nc.dram_tensor("wf1t", [D, D], F32, kind="ExternalInput")
    bf1r_in = nc.dram_tensor("bf1r", [1, D], F32, kind="ExternalInput")
    wf2t_in = nc.dram_tensor("wf2t", [D, OUT], F32, kind="ExternalInput")
    bf2r_in = nc.dram_tensor("bf2r", [1, OUT], F32, kind="ExternalInput")
    iota_in = nc.dram_tensor("iota", [P, P], BF16, kind="ExternalInput")
    identf_in = nc.dram_tensor("identf", [P, P], F32, kind="ExternalInput")
    onesg_in = nc.dram_tensor("onesg", [1, NC * GPC], F32, kind="ExternalInput")
    out = nc.dram_tensor("out", [NC * GPC, OUT], F32, kind="ExternalOutput")

    with tile.TileContext(nc) as tc:
        cp = tc.alloc_tile_pool(name="const", bufs=1)
        mp = tc.alloc_tile_pool(name="msgs", bufs=12)
        ohp = tc.alloc_tile_pool(name="ohp", bufs=4)
        wp = tc.alloc_tile_pool(name="work", bufs=3)
        pp_agg = tc.alloc_tile_pool(name="ps_agg", bufs=4, space="PSUM")
        pp_h = tc.alloc_tile_pool(name="ps_h", bufs=2, space="PSUM")
        dp = tc.alloc_tile_pool(name="dram", bufs=1, space="DRAM")

        h_loc = dp.tile([NPC, D], BF16)
        h_tabs = [dp.tile([QUAD, D], BF16, addr_space="Shared", name=f"htab{c}")
                  for c in range(NQ)]
        pag_in = dp.tile([P, GPC], F32)
        pag_out = dp.tile([NC * P, GPC], F32, addr_space="Shared")

        idx_t = cp.tile([P, S_tot * 8], I16)
        nc.sync.dma_start(idx_t[:], idx_in[:])
        dst_t = cp.tile([P, S_tot], BF16)
        nc.sync.dma_start(dst_t[:], dst_in[:])
        recip_t = cp.tile([P, NPC], BF16)
        nc.sync.dma_start(recip_t[:], recip_in[:])
        mask_t = cp.tile([1, NPC], BF16)
        nc.sync.dma_start(mask_t[:], mask_in[:])
        w1t_t = cp.tile([D, D], BF16)
        nc.sync.dma_start(w1t_t[:], w1t_in[:])
        w2t_t = cp.tile([D, D], BF16)
        nc.sync.dma_start(w2t_t[:], w2t_in[:])
        b1r_t = cp.tile([1, D], BF16)
        nc.sync.dma_start(b1r_t[:], b1r_in[:])
        b2r_t = cp.tile([1, D], BF16)
        nc.sync.dma_start(b2r_t[:], b2r_in[:])
        wf1t_t = cp.tile([D, D], F32)
        nc.sync.dma_start(wf1t_t[:], wf1t_in[:])
        bf1r_t = cp.tile([1, D], F32)
        nc.sync.dma_start(bf1r_t[:], bf1r_in[:])
        wf2t_t = cp.tile([D, OUT], F32)
        nc.sync.dma_start(wf2t_t[:], wf2t_in[:])
        bf2r_t = cp.tile([1, OUT], F32)
        nc.sync.dma_start(bf2r_t[:], bf2r_in[:])
        iota_t = cp.tile([P, P], BF16)
        nc.sync.dma_start(iota_t[:], iota_in[:])
        identf_t = cp.tile([P, P], F32)
        nc.sync.dma_start(identf_t[:], identf_in[:])
        onesg_t = cp.tile([1, NC * GPC], F32)
        nc.sync.dma_start(onesg_t[:], onesg_in[:])
        poolT = cp.tile([P, GPC], F32)
        nc.vector.memset(poolT[:], 0)
        # SBUF f32 window accumulators, transposed layout [feat, 104*128 dst]
        aggsb = cp.tile([P, WIN * P], BF16)

        # per-quad call schedule: subtiles sub0[q,0] .. sub0[q,0]+Sq
        qstart = [int(sub0[qi, 0]) for qi in range(NQ)]
        qcount = [int(S[qi].sum()) for qi in range(NQ)]

        call_no = [0]

        def epilogue(w, layer):
            """Normalize window w, apply weights; L1 -> h rows; L2 -> pool."""
            a = aggsb[:, w * P:(w + 1) * P]
            an = wp.tile([P, P], BF16, tag="an")
            nc.vector.tensor_tensor(
                out=an[:], in0=a,
                in1=recip_t[:, w * P:(w + 1) * P],
                op=mybir.AluOpType.mult)
            h_p = pp_h.tile([P, D], F32, tag="h")
            if layer == 0:
                nc.tensor.matmul(h_p[:], lhsT=an[:], rhs=w1t_t[:], start=True, stop=False)
                nc.tensor.matmul(h_p[:], lhsT=mask_t[:1, w * P:(w + 1) * P],
                                 rhs=b1r_t[:1, :], start=False, stop=True)
                h_s = wp.tile([P, D], BF16, tag="hs")
                nc.scalar.activation(h_s[:], h_p[:], mybir.ActivationFunctionType.Relu)
                nc.sync.dma_start(h_loc[w * P:(w + 1) * P, :], h_s[:])
            else:
                nc.tensor.matmul(h_p[:], lhsT=w2t_t[:], rhs=an[:], start=True, stop=False)
                nc.tensor.matmul(h_p[:], lhsT=b2r_t[:1, :],
                                 rhs=mask_t[:1, w * P:(w + 1) * P], start=False, stop=True)
                h_s = wp.tile([P, D], BF16, tag="hs")
                nc.scalar.activation(h_s[:], h_p[:], mybir.ActivationFunctionType.Relu)
                lg = w // WPG
                wmax = wp.tile([P, 1], F32, tag="wmax")
                nc.vector.reduce_max(wmax[:], h_s[:], axis=mybir.AxisListType.X)
                nc.vector.tensor_tensor(out=poolT[:, lg:lg + 1], in0=wmax[:],
                                        in1=poolT[:, lg:lg + 1],
                                        op=mybir.AluOpType.max)

        def run_layer(table, layer):
            for qi in range(NQ):
                # subtile schedule of this quad: list of (w, s_in_window, gs)
                sched = []
                for w in range(WIN):
                    for si in range(int(S[qi, w])):
                        sched.append((w, si, int(sub0[qi, w]) + si))
                # zero/epilogue for empty windows
                for w in range(WIN):
                    if int(S[qi, w]) == 0:
                        if qi == 0:
                            nc.vector.memset(aggsb[:, w * P:(w + 1) * P], 0)
                        if qi == NQ - 1:
                            epilogue(w, layer)
                pos = 0
                oh_cur = [None, -1]  # (o3 view, window)
                agg_cur = [None]
                pend = [None]

                def finish_window(w, agg_p):
                    dstslice = aggsb[:, w * P:(w + 1) * P]
                    if qi == 0:
                        nc.vector.tensor_copy(dstslice, agg_p[:])
                    else:
                        nc.vector.tensor_tensor(out=dstslice, in0=agg_p[:],
                                                in1=dstslice,
                                                op=mybir.AluOpType.add)
                    if qi == NQ - 1:
                        epilogue(w, layer)
                        if layer == 0 and w >= WCH + 7 and (w - WCH - 7) % WCH == 0:
                            c = (w - WCH - 7) // WCH
                            if c < NQ - 1:
                                nc.gpsimd.collective_compute(
                                    "AllGather", mybir.AluOpType.bypass,
                                    replica_groups=[list(range(NC))],
                                    ins=[h_loc[c * CPN:(c + 1) * CPN, :]],
                                    outs=[h_tabs[c][:, :]],
                                )

                while pos < len(sched):
                    n = min(MAXSUB, len(sched) - pos)
                    gs0 = sched[pos][2]
                    mg = mp.tile([P, MAXSUB * D], BF16, tag="msgs")
                    m3 = mg[:].rearrange("p (s d) -> p s d", d=D)
                    tab_ap = (table[qi * QUAD:(qi + 1) * QUAD, :]
                              if layer == 0 else h_tabs[qi][:, :])
                    nc.gpsimd.dma_gather(
                        out_ap=m3[:, :n, :],
                        in_ap=tab_ap,
                        idxs_ap=idx_t[:, gs0 * 8: (gs0 + n) * 8],
                        num_idxs=n * P,
                        num_idxs_reg=n * P,
                        elem_size=D,
                        single_packet=False,
                        queue_num=call_no[0] % 4,
                    )
                    call_no[0] += 1
                    if NOCOMPUTE:
                        pos += n
                        continue
                    for j in range(n):
                        w, si, gs = sched[pos + j]
                        Sqw = int(S[qi, w])
                        if si == 0:
                            g0 = int(sub0[qi, w])
                            oh = ohp.tile([P, Sqw * P], BF16, tag="oh",
                                          padded_shape=[P, MAXSUB * P])
                            o3 = oh[:].rearrange("p (s c) -> p s c", c=P)
                            nc.vector.tensor_tensor(
                                out=o3[:, :, :],
                                in0=dst_t[:, g0:g0 + Sqw].rearrange("p (s one) -> p s one", one=1)
                                    .to_broadcast([P, Sqw, P]),
                                in1=iota_t[:].rearrange("p (s c) -> p s c", c=P)
                                    .to_broadcast([P, Sqw, P]),
                                op=mybir.AluOpType.is_equal)
                            # flush/epilogue of the PREVIOUS window goes out
                            # after this one-hot: breaks the DVE->PE ping-pong
                            if pend[0] is not None:
                                finish_window(*pend[0])
                                pend[0] = None
                            oh_cur[0], oh_cur[1] = o3, w
                            agg_cur[0] = pp_agg.tile([P, P], F32, tag="agg", name="aggp")
                        o3w = oh_cur[0]
                        agg_p = agg_cur[0]
                        nc.tensor.matmul(agg_p[:], lhsT=m3[:, j, :], rhs=o3w[:, si, :],
                                         start=(si == 0), stop=(si == Sqw - 1))
                        if si == Sqw - 1:
                            pend[0] = (w, agg_p)
                    pos += n
                if pend[0] is not None:
                    finish_window(*pend[0])
                    pend[0] = None

        for layer in range(2):
            table = xt
            run_layer(table, layer)
            if layer == 0:
                c = NQ - 1
                nc.gpsimd.collective_compute(
                    "AllGather", mybir.AluOpType.bypass,
                    replica_groups=[list(range(NC))],
                    ins=[h_loc[c * CPN:(c + 1) * CPN, :]],
                    outs=[h_tabs[c][:, :]],
                )

        # ---- head ----
        NGr = NC * GPC
        nc.sync.dma_start(pag_in[:], poolT[:])
        nc.gpsimd.collective_compute(
            "AllGather", mybir.AluOpType.bypass,
            replica_groups=[list(range(NC))],
            ins=[pag_in[:]], outs=[pag_out[:]],
        )
        pall = cp.tile([P, NGr], F32)
        pr = pag_out[:].rearrange("(c p) g -> p c g", c=NC)
        for c in range(NC):
            nc.sync.dma_start(pall[:, c * GPC:(c + 1) * GPC], pr[:, c, :])
        z_p = pp_agg.tile([P, NGr], F32, tag="agg")
        nc.tensor.matmul(z_p[:], lhsT=wf1t_t[:], rhs=pall[:], start=True, stop=False)
        nc.tensor.matmul(z_p[:], lhsT=bf1r_t[:1, :], rhs=onesg_t[:1, :], start=False, stop=True)
        zr = wp.tile([P, NGr], F32, tag="zr")
        nc.scalar.activation(zr[:], z_p[:], mybir.ActivationFunctionType.Relu)
        z2_p = pp_h.tile([OUT, NGr], F32, tag="h")
        nc.tensor.matmul(z2_p[:], lhsT=wf2t_t[:], rhs=zr[:], start=True, stop=False)
        nc.tensor.matmul(z2_p[:], lhsT=bf2r_t[:1, :], rhs=onesg_t[:1, :], start=False, stop=True)
        z2 = wp.tile([OUT, NGr], F32, tag="z2")
        nc.vector.tensor_copy(z2[:], z2_p[:])
        zt_p = pp_agg.tile([NGr, OUT], F32, tag="agg")
        nc.tensor.transpose(zt_p[:], z2[:], identf_t[:OUT, :OUT])
        zt = wp.tile([NGr, OUT], F32, tag="zt")
        nc.vector.tensor_copy(zt[:], zt_p[:])
        mx = wp.tile([NGr, 1], F32, tag="mx")
        nc.vector.reduce_max(mx[:], zt[:], axis=mybir.AxisListType.X)
        zs = wp.tile([NGr, OUT], F32, tag="zs")
        nc.vector.tensor_scalar(out=zs[:], in0=zt[:], scalar1=mx[:], scalar2=None,
                                op0=mybir.AluOpType.subtract)
        ex = wp.tile([NGr, OUT], F32, tag="ex")
        nc.scalar.activation(ex[:], zs[:], mybir.ActivationFunctionType.Exp)
        sm = wp.tile([NGr, 1], F32, tag="sm")
        nc.vector.reduce_sum(sm[:], ex[:], axis=mybir.AxisListType.X)
        lg_ = wp.tile([NGr, 1], F32, tag="lg")
        nc.scalar.activation(lg_[:], sm[:], mybir.ActivationFunctionType.Ln)
        logz = wp.tile([NGr, 1], F32, tag="logz")
        nc.vector.tensor_tensor(out=logz[:], in0=mx[:], in1=lg_[:],
                                op=mybir.AluOpType.add)
        res = wp.tile([NGr, OUT], F32, tag="res")
        nc.vector.tensor_scalar(out=res[:], in0=zt[:], scalar1=logz[:], scalar2=None,
                                op0=mybir.AluOpType.subtract)
        nc.sync.dma_start(out[:], res[:])

        for p_ in (dp, pp_h, pp_agg, wp, ohp, mp, cp):
            p_.release()
    nc.compile()
    return nc


def make_inputs(st, percore, W1, b1, W2, b2, Wf1, bf1, Wf2, bf2):
    GPC = st["GPC"]
    bf = ml_dtypes.bfloat16
    iota = np.broadcast_to(np.arange(P, dtype=np.float32), (P, P)).astype(bf)
    ident = np.eye(P, dtype=np.float32)
    common = dict(
        xt=percore["xt"],
        w1t=np.ascontiguousarray(np.asarray(W1, np.float32).T).astype(bf),
        w2t=np.ascontiguousarray(np.asarray(W2, np.float32).T).astype(bf),
        b1r=np.asarray(b1, np.float32)[None, :].astype(bf),
        b2r=np.asarray(b2, np.float32)[None, :].astype(bf),
        wf1t=np.ascontiguousarray(np.asarray(Wf1, np.float32).T),
        bf1r=np.asarray(bf1, np.float32)[None, :],
        wf2t=np.ascontiguousarray(np.asarray(Wf2, np.float32).T),
        bf2r=np.asarray(bf2, np.float32)[None, :],
        iota=np.ascontiguousarray(iota),
        identf=ident,
        onesg=np.ones((1, NC * GPC), np.float32),
    )
    in_maps = []
    for c in range(NC):
        m = dict(common)
        m["idx16"] = np.ascontiguousarray(percore["idx16"][c])
        m["dstl"] = np.ascontiguousarray(percore["dst"][c])
        m["recip"] = np.ascontiguousarray(percore["recip"][c])
        m["mask"] = np.ascontiguousarray(percore["mask"][c])
        in_maps.append(m)
    return in_maps


_CACHE = {}


def kernel(**inputs):
    """Full-input GNN kernel: shards across 8 NeuronCores internally."""
    import os
    x = np.asarray(inputs["x"], np.float32)
    ei = np.asarray(inputs["edge_index"])
    batch = np.asarray(inputs["batch"])
    st, percore = preprocess(x, ei, batch)
    key = (st["WIN"], st["NPC"], st["S_tot"], st["NQ"])
    if key not in _CACHE:
        _CACHE[key] = build_nc(st)
    nc = _CACHE[key]
    in_maps = make_inputs(st, percore,
                          inputs["W1"], inputs["b1"], inputs["W2"], inputs["b2"],
                          inputs["Wf1"], inputs["bf1"], inputs["Wf2"], inputs["bf2"])
    trace = os.environ.get("GNN_TRACE", "0") == "1"
    res = run_bass_kernel_spmd(nc, in_maps, core_ids=list(range(NC)), trace=trace)
    global LAST_EXEC_NS, LAST_TRACE
    LAST_EXEC_NS = res.exec_time_ns
    LAST_TRACE = res.instructions_and_trace[1] if res.instructions_and_trace else None
    return np.asarray(res.results[0]["out"], np.float32)


LAST_EXEC_NS = None
LAST_TRACE = None
